# revision 26
# baseline (speedup 1.0000x reference)
"""Trainium2 Bass kernel for nn_Attention_45148696216391.

Multi-head attention with QK L2-norm (qk-norm) + learned per-head scales:
  q = x @ Wq.T ; k = x @ Wk.T ; v = x @ Wv.T       (per head, dh=64)
  q = l2norm(q) * q_scale ; k = l2norm(k) * k_scale
  out = softmax(q k^T / sqrt(dh)) @ v ; out = out @ Wo.T + bo

Sharding (8 cores): data parallel over batch b (2) x tensor parallel over
heads (16 heads -> 4 per core).  Each core computes, for its (b, head-group):
    P_out^T = Wo_s^T @ O^T   in (d, n) layout  -- a PARTIAL sum over e-dims.
Host reduces the 4 head-group partials per batch, transposes, adds bo.

Per-core dataflow (everything transposed, d/e on partitions; bf16 matmul
operands, fp32 PSUM accumulation):
  xt (1024, 2048) = x[b].T streamed in [128, 512] tiles.
  Q^T/K^T per (head-PAIR, i512-block) in [128, 512] tiles: rows 0:64 =
  head 2c, rows 64:128 = head 2c+1 (no zero padding).  Scores use PE
  row-tiling: two concurrent K=64 matmuls via tile_position (0,0)/(64,0)
  into separate PSUM banks -- both heads of a pair stream together, so
  the full 128x128 array stays busy and the score stream takes ~half the
  cycles of the padded-K=128 variant.
  q_scale/sqrt(dh), k_scale are folded into the weights host-side; the
  l2-norm 'undoes' them via a 1/s^2-valued reduction mask (ss = mask.T@q'^2).
  Square on DVE (keeps ACT free for exp), Sqrt on ACT, 1/x via
  reciprocal_approx_fast, replicated across partitions with
  gpsimd.partition_broadcast (no DRAM bounce).
  V natural per j-chunk in [128, 4*128] tiles: per head 64 V cols + a ones
  col (makes the PV matmul also emit the softmax denominator Z).
  scores S^T[j, i] in psum pairs [128, 1024] (two j-tiles) -> exp on ACT
  (no max subtraction: q,k unit vectors so |s| <= q_scale*k_scale/8) ->
  PV accumulates O^T[dh + Z + pad, i] over 16 j-tiles.
  epilogue: O^T copied to SBUF (frees the psum bank fast), 1/Z via DVE
  recip + gpsimd partition_broadcast, one multiply into OC.
  out-proj per i512: psum[d-tile, i] = sum_ec WoT[ec] @ O^T[ec] -> DRAM.

Emission order is a fine-grained interleave so exp (the ACT bottleneck,
~16.8M elements) starts after ~1/4 of the projection work and runs
continuously: K/Q/V projections per i5 feed attention j-chunk groups as
soon as their tiles exist; ec1 projections and out-projs hide inside the
ACT-bound attention stream; outproj(i5) follows att(i5,1) immediately so
only the last block's epilogue + outproj trail the final exp.
"""

import os
import sys

sys.path.insert(0, "/opt/trn_rl_repo")

import numpy as np

import concourse.bacc as bacc
import concourse.mybir as mybir
import concourse.tile as tile
from concourse import library_config

B, N, DIM = 2, 2048, 1024
H, DH = 16, 64
E = 256            # inner dims per core (4 heads x 64)
NC = 8             # cores
HPC = 4            # heads per core
I512 = 512         # i-tile
NI = N // I512     # 4 i-blocks
NDC = DIM // 128   # 8 d-chunks
NJT = N // 128     # 16 j-tiles

f32 = mybir.dt.float32
f32r = mybir.dt.float32r
bf16 = mybir.dt.bfloat16

# matmul operand dtype: bf16 (full PE rate, FWL, HAM warms) | f32r | f32
MM_DT = os.environ.get("KMM_DT", "bf16")
MMD = {"bf16": bf16, "f32r": f32r, "f32": f32}[MM_DT]


def _act_set_id(arch):
    """Index of the ACT table set containing exp+ln+square
    (natural_log_exp_and_others; id 6 for gen3/TRN2)."""
    from concourse.hw_specs import get_activation_tables

    need = {
        mybir.ActivationFunctionType.Exp,
        mybir.ActivationFunctionType.Ln,
        mybir.ActivationFunctionType.Square,
    }
    for idx, (name, fns) in enumerate(get_activation_tables(arch).items()):
        if need <= fns:
            return idx
    raise RuntimeError("no ACT table set with exp+ln+square")


def build_nc():
    nc = bacc.Bacc("TRN2", target_bir_lowering=False, debug=False)

    xt = nc.dram_tensor("xt", [DIM, N], MMD, kind="ExternalInput").ap()
    wqt = nc.dram_tensor("wqt", [DIM, E], MMD, kind="ExternalInput").ap()
    wkt = nc.dram_tensor("wkt", [DIM, E], MMD, kind="ExternalInput").ap()
    wvt = nc.dram_tensor("wvt", [DIM, E], MMD, kind="ExternalInput").ap()
    wot = nc.dram_tensor("wot", [E, DIM], MMD, kind="ExternalInput").ap()
    hmk = nc.dram_tensor("hmk", [128, 66], MMD, kind="ExternalInput").ap()
    # norm masks: col 0 -> head A (out partition 0), col 64 -> head B (out
    # partition 64) so every later partition offset is 0/64 (32-granular)
    nmq = nc.dram_tensor("nmq", [128, 2, 65], MMD, kind="ExternalInput").ap()
    nmk = nc.dram_tensor("nmk", [128, 2, 65], MMD, kind="ExternalInput").ap()
    # output split by e-chunk: host sums out0+out1 (it already reduces the
    # 4 head-group partials), so each half out-projection only depends on
    # one OC stream and can be scheduled as soon as that stream's epilogue
    # is done
    out0 = nc.dram_tensor("out0", [DIM, N], f32, kind="ExternalOutput").ap()
    out1 = nc.dram_tensor("out1", [DIM, N], f32, kind="ExternalOutput").ap()
    outs = [out0, out1]

    with tile.TileContext(nc) as tc:
        with (
            tc.tile_pool(name="wpool", bufs=1) as wpool,
            tc.tile_pool(name="big", bufs=1) as big,
            tc.tile_pool(name="xts", bufs=4) as xts,
            tc.tile_pool(name="sqp", bufs=4) as sqp,
            tc.tile_pool(name="nsp", bufs=6) as nsp,
            tc.tile_pool(name="ptp", bufs=8) as ptp,
            tc.tile_pool(name="obp", bufs=3) as obp,
            tc.tile_pool(name="pa", bufs=3, space="PSUM") as pa,
            tc.tile_pool(name="po", bufs=2, space="PSUM") as po,
        ):
            # gpsimd extended-instruction library for partition_broadcast
            nc.gpsimd.load_library(library_config.attn)
            # pin the ACT table set that holds ALL functions we use
            # (exp, ln, square) so the compiler's table-load pass never
            # inserts another ~1.5us ACT_TABLE_LOAD mid-kernel
            nc.scalar.add_instruction(
                mybir.InstLoadActFuncSet(
                    name=nc.get_next_instruction_name(),
                    ins=[],
                    outs=[],
                    act_func_set_id=_act_set_id(nc.m.arch),
                )
            )

            # ---- weights + constants in SBUF ----
            WQT = wpool.tile([128, NDC, E], MMD)  # [d_in_chunk, dc, e]
            WKT = wpool.tile([128, NDC, E], MMD)
            WVT = wpool.tile([128, NDC, E], MMD)
            WOT = wpool.tile([128, 2, DIM], MMD)  # [e_in_chunk, ec, d]
            # WKT per-chunk so the first projection starts after 64KB
            for dc in range(NDC):
                nc.sync.dma_start(WKT[:, dc, :], wkt[128 * dc : 128 * (dc + 1), :])
            nc.sync.dma_start(WQT[:], wqt.rearrange("(dc p) e -> p dc e", p=128))
            nc.sync.dma_start(WVT[:], wvt.rearrange("(dc p) e -> p dc e", p=128))
            nc.sync.dma_start(WOT[:], wot.rearrange("(ec p) d -> p ec d", p=128))
            HM = wpool.tile([128, 66], MMD)  # cols 0-1: head mask; 2-65: ones
            nc.sync.dma_start(HM[:], hmk)
            # norm-reduction masks with 1/s^2 folded in: ss = mask.T @ q'^2
            # recovers ||q||^2 of the unscaled q (weights carry s)
            NMQ = wpool.tile([128, 2, 65], MMD)
            NMK = wpool.tile([128, 2, 65], MMD)
            nc.sync.dma_start(NMQ[:], nmq)
            nc.sync.dma_start(NMK[:], nmk)

            # ---- per-block persistent tiles (independent dataflow units) ----
            # QT/KT per (head-pair c, i5): rows 0:64 head 2c, 64:128 head 2c+1
            QT = [
                [big.tile([128, I512], MMD, name=f"qt{c}_{i}", tag=f"qt{c}_{i}")
                 for i in range(NI)]
                for c in range(2)
            ]
            KT = [
                [big.tile([128, I512], MMD, name=f"kt{c}_{i}", tag=f"kt{c}_{i}")
                 for i in range(NI)]
                for c in range(2)
            ]
            OC = [
                [big.tile([128, I512], MMD, name=f"oc{c}_{i}", tag=f"oc{c}_{i}")
                 for i in range(NI)]
                for c in range(2)
            ]
            VA = [
                big.tile([128, HPC * 128], MMD, name=f"va{j}", tag=f"va{j}")
                for j in range(NJT)
            ]
            # persistent Z staging tile: rows 0/64 carry the two heads' Z,
            # rows 1-63 memset once so the full-tile recip reads no uninit
            ZROW = big.tile([65, I512], f32, name="zrowp", tag="zrowp")
            nc.vector.memset(ZROW[:], 0.0)
            for j in range(NJT):
                nc.gpsimd.memset(VA[j][:], 0.0)
                nc.vector.tensor_copy(
                    VA[j].rearrange("p (h c) -> p h c", c=128)[:, :, 64:65],
                    HM[:, 2:3].to_broadcast([128, HPC, 1]),
                )

            # ---- x in SBUF ----
            xtls = []
            for i5 in range(NI):
                isl = slice(i5 * I512, (i5 + 1) * I512)
                xb = xts.tile([128, NDC, I512], MMD, tag="xt", name=f"xb{i5}")
                if i5 == 0:
                    # first block: per-chunk DMAs so the very first matmul
                    # starts after 128KB instead of 1MB
                    for dc in range(NDC):
                        nc.sync.dma_start(
                            xb[:, dc, :], xt[128 * dc : 128 * (dc + 1), isl]
                        )
                else:
                    nc.sync.dma_start(
                        xb[:], xt.rearrange("(dc p) n -> p dc n", p=128)[:, :, isl]
                    )
                xtls.append([xb[:, dc, :] for dc in range(NDC)])

            def qk_proj(i5, ec, WT, NM, DST):
                xtl = xtls[i5]
                pq = pa.tile([128, I512], f32, tag="A", name="pq")
                for dc in range(NDC):
                    nc.tensor.matmul(
                        pq[:],
                        WT[:, dc, 128 * ec : 128 * (ec + 1)],
                        xtl[dc][:],
                        start=(dc == 0),
                        stop=(dc == NDC - 1),
                    )
                # the 1/s^2 descale rides in the reduction mask
                sq = sqp.tile([128, I512], MMD, tag="sq")
                nc.scalar.activation(
                    sq[:], pq[:], mybir.ActivationFunctionType.Square
                )
                pnn = po.tile([65, I512], f32, tag="po", name="pnn")
                nc.tensor.matmul(
                    pnn[:], NM[:, ec, :], sq[:], start=True, stop=True
                )
                # 1/sqrt(ss) = exp(-0.5*ln(ss)): Ln/Exp/Square share one ACT
                # table set, so no mid-kernel ACT table reloads (Sqrt's set
                # lacks exp and forced a ~1.3us reload per projection)
                ns = nsp.tile([65, I512], f32, tag="ns")
                nc.scalar.activation(
                    ns[:], pnn[:], mybir.ActivationFunctionType.Ln
                )
                rq = nsp.tile([65, I512], f32, tag="rq")
                nc.scalar.activation(
                    rq[:], ns[:], mybir.ActivationFunctionType.Exp, scale=-0.5
                )
                # partition_broadcast's Q7 kernel reads the source on core 0
                # (partitions 0-15), so head B's row must be copied to a
                # base-partition-0 tile first
                rqb = nsp.tile([1, I512], f32, tag="rqb")
                nc.vector.tensor_copy(rqb[:], rq[64:65, :])
                for hh, src in ((0, rq[0:1, :]), (1, rqb[:])):
                    rr = sqp.tile([64, I512], f32, tag="rr")
                    nc.gpsimd.partition_broadcast(rr[:], src)
                    nc.vector.tensor_tensor(
                        DST[ec][i5][64 * hh : 64 * hh + 64, :],
                        pq[64 * hh : 64 * hh + 64, :],
                        rr[:],
                        mybir.AluOpType.mult,
                    )

            def v_proj(nt):
                i5, ntl = divmod(nt, 4)
                pv = pa.tile([128, E], f32, tag="A", name="pv")
                for dc in range(NDC):
                    nc.tensor.matmul(
                        pv[:],
                        xtls[i5][dc][:, 128 * ntl : 128 * (ntl + 1)],
                        WVT[:, dc, :],
                        start=(dc == 0),
                        stop=(dc == NDC - 1),
                    )
                nc.vector.tensor_copy(
                    VA[nt].rearrange("p (h c) -> p h c", c=128)[:, :, 0:64],
                    pv[:].rearrange("p (h c) -> p h c", c=64),
                )

            # ---- attention: software-pipelined scores/exp vs PV ----
            # The PE queue is in-order; if PV(jp) (which waits on exp(jp))
            # sits right behind scores(jp), the queue stalls on ACT and the
            # exp stream starves in turn.  Emitting PV one jp late keeps
            # independent score matmuls in front of the PE at all times.
            pos_live = {}

            def att_scores(i5, c, jp):
                pscs = [
                    pa.tile([128, 1024], f32, tag="A", name=f"psc{_d}")
                    for _d in range(2)
                ]
                for d in range(2):
                    for u in range(2):
                        jt = 2 * jp + u
                        # row-tiled: head d runs in PE rows 64d..64d+63
                        nc.tensor.matmul(
                            pscs[d][:, 512 * u : 512 * (u + 1)],
                            KT[c][jt // 4][
                                64 * d : 64 * (d + 1),
                                128 * (jt % 4) : 128 * (jt % 4) + 128,
                            ],
                            QT[c][i5][64 * d : 64 * (d + 1), :],
                            start=True,
                            stop=True,
                            tile_position=(64 * d, 0),
                        )
                pts = []
                for d in range(2):
                    pt = ptp.tile([128, 1024], MMD, tag="pt")
                    nc.scalar.activation(
                        pt[:], pscs[d][:], mybir.ActivationFunctionType.Exp
                    )
                    pts.append(pt)
                return pts

            def att_pv(i5, c, jp, pts):
                if (i5, c) not in pos_live:
                    pos_live[(i5, c)] = [
                        po.tile([128, I512], f32, tag="po", name=f"po{_d}")
                        for _d in range(2)
                    ]
                pos = pos_live[(i5, c)]
                for d in range(2):
                    h = 2 * c + d
                    for u in range(2):
                        jt = 2 * jp + u
                        nc.tensor.matmul(
                            pos[d][:],
                            VA[jt][:, 128 * h : 128 * h + 128],
                            pts[d][:, 512 * u : 512 * (u + 1)],
                            start=(jt == 0),
                            stop=(jt == NJT - 1),
                        )

            def att_block(i5, c, vfeed=False):
                prev = None
                for jp in range(NJT // 2):
                    pts = att_scores(i5, c, jp)
                    if prev is not None:
                        att_pv(i5, c, jp - 1, prev)
                    if vfeed and jp < 7:
                        v_proj(2 * jp + 2)
                        v_proj(2 * jp + 3)
                    prev = pts
                att_pv(i5, c, NJT // 2 - 1, prev)

            def att_epi(i5, c):
                pos = pos_live.pop((i5, c))
                # O^T + Z to SBUF fast (frees the po banks), 1/Z via DVE
                # recip (needs base partition 0) + gpsimd partition bcast
                ots = []
                for d in range(2):
                    ot = nsp.tile([64, I512], f32, tag="ot")
                    nc.vector.tensor_copy(ot[:], pos[d][0:64, :])
                    nc.vector.tensor_copy(
                        ZROW[64 * d : 64 * d + 1, :], pos[d][64:65, :]
                    )
                    ots.append(ot)
                rz = nsp.tile([65, I512], f32, tag="rz")
                nc.vector.reciprocal_approx_fast(rz[:], ZROW[:])
                rzb = nsp.tile([1, I512], f32, tag="rqb")
                nc.vector.tensor_copy(rzb[:], rz[64:65, :])
                for d, src in ((0, rz[0:1, :]), (1, rzb[:])):
                    rzr = sqp.tile([64, I512], f32, tag="rr")
                    nc.gpsimd.partition_broadcast(rzr[:], src)
                    nc.vector.tensor_tensor(
                        OC[c][i5][64 * d : 64 * (d + 1), :],
                        ots[d][:],
                        rzr[:],
                        mybir.AluOpType.mult,
                    )

            def outproj_half(ec, i5):
                isl = slice(i5 * I512, (i5 + 1) * I512)
                for dt in range(NDC):
                    pp_o = pa.tile([128, I512], f32, tag="A", name="ppo")
                    nc.tensor.matmul(
                        pp_o[:],
                        WOT[:, ec, 128 * dt : 128 * (dt + 1)],
                        OC[ec][i5][:],
                        start=True,
                        stop=True,
                    )
                    ob = obp.tile([128, I512], f32, tag="ob")
                    nc.vector.tensor_copy(ob[:], pp_o[:])
                    nc.sync.dma_start(
                        outs[ec][128 * dt : 128 * (dt + 1), isl], ob[:]
                    )

            # ---- emission schedule ----
            # Minimal prologue before the first exp: K projections (scores
            # need K over all j), Q(0), and the first two V tiles.  The
            # remaining V tiles feed in between att(0,0) units; qk_proj and
            # outproj halves sit at block boundaries (pos-free windows, so
            # pnn can share the po pool without deadlock).  Each boundary
            # also pre-projects Q/K one block ahead of its consumer.
            for i5 in range(NI):
                qk_proj(i5, 0, WKT, NMK, KT)
            qk_proj(0, 0, WQT, NMQ, QT)
            v_proj(0)
            v_proj(1)
            att_block(0, 0, vfeed=True)
            att_epi(0, 0)
            qk_proj(1, 0, WQT, NMQ, QT)
            outproj_half(0, 0)
            att_block(1, 0)
            att_epi(1, 0)
            qk_proj(2, 0, WQT, NMQ, QT)
            qk_proj(0, 1, WKT, NMK, KT)
            outproj_half(0, 1)
            att_block(2, 0)
            att_epi(2, 0)
            qk_proj(3, 0, WQT, NMQ, QT)
            qk_proj(1, 1, WKT, NMK, KT)
            outproj_half(0, 2)
            att_block(3, 0)
            att_epi(3, 0)
            qk_proj(2, 1, WKT, NMK, KT)
            qk_proj(3, 1, WKT, NMK, KT)
            qk_proj(0, 1, WQT, NMQ, QT)
            outproj_half(0, 3)
            att_block(0, 1)
            att_epi(0, 1)
            qk_proj(1, 1, WQT, NMQ, QT)
            outproj_half(1, 0)
            att_block(1, 1)
            att_epi(1, 1)
            qk_proj(2, 1, WQT, NMQ, QT)
            outproj_half(1, 1)
            att_block(2, 1)
            att_epi(2, 1)
            qk_proj(3, 1, WQT, NMQ, QT)
            outproj_half(1, 2)
            att_block(3, 1)
            att_epi(3, 1)
            outproj_half(1, 3)

    nc.compile()
    return nc


def make_in_maps(x, Wq, Wk, Wv, Wo, q_scale, k_scale):
    """Shard + lay out the full inputs for the 8 cores."""
    npdt = mybir.dt.np(MMD)
    x = np.asarray(x, dtype=np.float32)
    Wq = np.asarray(Wq, dtype=np.float32)
    Wk = np.asarray(Wk, dtype=np.float32)
    Wv = np.asarray(Wv, dtype=np.float32)
    Wo = np.asarray(Wo, dtype=np.float32)
    qs = np.asarray(q_scale, dtype=np.float32).reshape(H, DH)
    ks = np.asarray(k_scale, dtype=np.float32).reshape(H, DH)

    hmk = np.zeros((128, 66), np.float32)
    hmk[0:64, 0] = 1.0
    hmk[64:128, 1] = 1.0
    hmk[:, 2:66] = 1.0

    xts_ = [np.ascontiguousarray(x[b].T).astype(npdt) for b in range(B)]
    hmk = hmk.astype(npdt)
    in_maps = []
    for core in range(NC):
        b, g = divmod(core, 4)
        esl = slice(E * g, E * (g + 1))
        qsv = qs[HPC * g : HPC * g + HPC].reshape(E) * DH ** -0.5  # (256,)
        ksv = ks[HPC * g : HPC * g + HPC].reshape(E)
        nmq = np.zeros((128, 2, 65), np.float32)
        nmk = np.zeros((128, 2, 65), np.float32)
        # unused mask cols 1..63 pick up sq[0,:] so the downstream Ln never
        # sees an exact 0 (rows 1-63 of the norm tile are dead but computed)
        nmq[0, :, 1:64] = 1.0
        nmk[0, :, 1:64] = 1.0
        for ec in range(2):
            for p in range(128):
                col = 0 if p < 64 else 64
                nmq[p, ec, col] = 1.0 / qsv[128 * ec + p] ** 2
                nmk[p, ec, col] = 1.0 / ksv[128 * ec + p] ** 2
        in_maps.append(
            {
                "xt": xts_[b],
                "wqt": np.ascontiguousarray(Wq[esl].T * qsv[None, :]).astype(npdt),
                "wkt": np.ascontiguousarray(Wk[esl].T * ksv[None, :]).astype(npdt),
                "wvt": np.ascontiguousarray(Wv[esl].T).astype(npdt),
                "wot": np.ascontiguousarray(Wo[:, esl].T).astype(npdt),
                "hmk": hmk,
                "nmq": nmq.astype(npdt),
                "nmk": nmk.astype(npdt),
            }
        )
    return in_maps


def gather_output(results, bo):
    """results: 8 dicts with 'out0'/'out1' (1024, 2048) partial^T arrays."""
    bo = np.asarray(bo, dtype=np.float32)
    out = np.empty((B, N, DIM), np.float32)
    for b in range(B):
        acc = results[4 * b]["out0"].astype(np.float32)
        acc = acc + results[4 * b]["out1"]
        for g in range(1, 4):
            acc = acc + results[4 * b + g]["out0"]
            acc = acc + results[4 * b + g]["out1"]
        out[b] = acc.T + bo
    return out


_NC_CACHE = {}


def kernel(x, Wq, Wk, Wv, Wo, bo, q_scale, k_scale):
    from concourse.bass_utils import run_bass_kernel_spmd

    key = MM_DT
    if key not in _NC_CACHE:
        _NC_CACHE[key] = build_nc()
    nc = _NC_CACHE[key]
    in_maps = make_in_maps(x, Wq, Wk, Wv, Wo, q_scale, k_scale)
    res = run_bass_kernel_spmd(nc, in_maps, list(range(NC)))
    return gather_output(res.results, bo)
